# revision 14
# baseline (speedup 1.0000x reference)
"""Trainium2 Bass kernel for BatchedCauchyKernel_CONCERT_flex (v3).

Full-input contract: kernel(**inputs) takes the complete (unsharded)
numpy arrays, shards x/sample_x/cutoff rows across 8 NeuronCores
(data-parallel over the N axis of the output), replicates y/sample_y/
scale, and gathers the per-core [512, 4096] tiles into the full
[4096, 4096] output.

Math (reference):
    s        = clip(scale, 1e-6, 1e6)
    scale_x  = clip(sample_x @ s, 1e-6)        x_s = x / sqrt(scale_x)
    scale_y  = clip(sample_y @ s, 1e-6)        y_s = y / sqrt(scale_y)
    d        = clip(|x_s_i|^2 + |y_s_j|^2 - 2 x_s_i . y_s_j, 1e-6)
    res      = 1 / (1 + d)
    c        = clip(cutoff, 1e-4, 0.9999)
    cm_ij    = (c_i + c_j) / 2
    out      = res * sigmoid(clip(res - cm, -1, 1))     (iff mean(cutoff) > 0)

Device formulation (per core) -- v2's math, v3's schedule:
    PSUM = 2 + 2|x_i|^2 + 2|y_j|^2 - 4 x.y = 2(1+d)  via accumulating
    matmuls (bf16 mains K=128 vs -4*ysT; split-bf16 aug K=4 for the
    doubled norm terms; two aug streams run concurrently at PE
    row-groups 0 and 32 via tile_position).
    ACT:  res' = Reciprocal(PSUM) -> bf16          [res' = res/2]
    DVE:  fused CAUCHY_GATE_ANT op (8 ALU nodes, 1 elem/cycle):
          t'  = Src0 - (Src1 + C0)                 [= (res - (c_j+c_i)/2)/2]
          out = Src0 * (One + t'*(C2 + C1*t'^2))   [cubic sigmoid fit]

v3 schedule changes (driven by the v2 trace; all windows in exec-time):
    - v2 spent ~25us of serialized DIRECT2D descriptor-gen on the Sync
      sequencer (41 dma_starts x ~605ns) -- input loading could not even
      begin until ~2.3us into the window and the PE sat idle for the
      first 6us. v3 cuts input dma_starts 25 -> 7 (2048-col ysT/cjb
      chunks = 4KB descriptor lines; aug tensors packed into one dram
      param loaded twice) and splits them across BOTH HWDGE rings
      (nc.scalar + nc.sync), with the 16 output DMAs alone on Sync.
    - PSUM is organized as two 4-bank pair-tiles [128, 2048] (pools
      psA/psB) instead of four 2-bank tiles: per pair, the 4 mains
      share one xsT LDWEIGHTS and the two K=4 aug streams cover 2048
      cols in one 2-way concurrent pass (was: LDW + aug pass per 1024
      cols). Steady-state PE cost drops ~1507 -> ~1340ns per 1024 cols
      at the DMA-throttled 1.2 GHz PE clock. ACT reads one pair while
      the PE fills the other (opposite PSUM halves, as in v2).
    - Epilogue still drains in 1024-col halves (ACT recip -> DVE gate
      -> out DMA) so ACT/DVE stay under the PE pace and the last-tile
      drain stays short.
    - Warmup is 16 x 128-col MMs off xsT (lands first) bridging the gap
      until ysT chunk 0 arrives -- keeps the PE continuously busy from
      ~2us into the window (v2: first MM at ~6us).
    - ACT spline-table preload via a dummy 1-element Reciprocal on xsT
      right after the ysT chunk-0 descriptor is issued.
    - Pool buffers cut 18 -> 9 (wresp/wot 6 -> 3) to shrink the
      end-of-program teardown (drain + barriers + dma_reset/sem_clear
      scale with allocated sems; v2 spent ~9us there).
The row scaling / row norms (O(N*D), 0.025% of the FLOPs) are host prep.
"""

from __future__ import annotations

import numpy as np

N = 4096
D = 128
S = 16
NCORES = 8
R = N // NCORES          # 512 rows of x per core
RCHUNKS = R // 128       # 4 row chunks of 128 (PSUM partition dim)
W = 1024                 # epilogue tile width (2 PSUM banks)
PAIRW = 2048             # PSUM pair-tile width (4 banks)

# sigmoid(t) ~= 0.5 + A1*t + A3*t^3, minimax on t in [-1, 0.05] (err 1.2e-4)
A1 = 0.24939704
A3 = -0.01842716

_PROGRAM_CACHE = {}
_GATE_OP = []


def _register_gate_op():
    """Register the fused gate op in concourse.dve_ops' tables (the
    documented extension point is appending to OPS; the per-NEFF DVE
    table is generated from OPS by name). Idempotent."""
    import concourse.dve_ops as dops
    from concourse.dve_spec import Spec, Src0, Src1, C0, C1, C2, One
    from concourse.dve_uop import DveOpSpec

    if _GATE_OP:
        return _GATE_OP[0]
    name = "CAUCHY_GATE_ANT"
    for op in dops.OPS:
        if op.name == name:
            _GATE_OP.append(op)
            return op

    s = Src1 + C0
    t = Src0 - s
    w = One + t * (C2 + C1 * (t * t))
    body = w * Src0

    def ref(in0, in1, c0, c1, c2):
        i0 = in0.astype(np.float32)
        tt = i0 - (in1.astype(np.float32) + c0)
        return (i0 * (1.0 + tt * (c2 + c1 * (tt * tt)))).astype(np.float32)

    spec = Spec(body=body, reference=ref)
    row = dops._CUSTOM_DVE_ROW_BASE + len(dops.OPS)
    assert row < 0x20
    tmp = DveOpSpec(
        name=name, opcode=row, uops=dops.lower(spec, ver="v3"), rd1_en=True
    )
    op = dops.DveOp(name, spec, subdim=False, uops_sha={"v3": tmp.sha("v3")})
    dops.OPS.append(op)
    dops._SUB_OPCODE_FOR_NAME[name] = row
    dops.CUSTOM_DVE_SPECS[name] = spec
    _GATE_OP.append(op)
    return op


def _build_program(apply_gate: bool):
    from contextlib import ExitStack

    import concourse.bass as bass
    import concourse.tile as tile
    from concourse import bacc, mybir

    f32 = mybir.dt.float32
    bf16 = mybir.dt.bfloat16
    gate_op = _register_gate_op()

    nc = bacc.Bacc()

    xsT_d = nc.declare_dram_parameter("xsT", [128, R], bf16, isOutput=False)
    ysT_d = nc.declare_dram_parameter("ysT", [128, N], bf16, isOutput=False)
    # aug rows k over cols [x-part(0:R) | y-part(R:R+N)]:
    #   k=0: [x2h | 1],  k=1: [x2l | 1],  k=2: [1 | yh],  k=3: [1 | yl]
    # so sum_k augx[k,i]*augy[k,j] = 2*x2_i + 2*y2_j + 2.
    aug_d = nc.declare_dram_parameter("aug", [4, R + N], bf16, isOutput=False)
    hci_d = nc.declare_dram_parameter("hci", [128, RCHUNKS], f32, isOutput=False)
    hcj_d = nc.declare_dram_parameter("hcj", [1, N], bf16, isOutput=False)
    out_d = nc.declare_dram_parameter("out", [R, N], bf16, isOutput=True)

    def raw_activation(out, in_, func, bias=0.0, scale=1.0, alpha=0.0):
        sc = nc.scalar
        inputs = [sc.lower_ap(in_)]
        for arg in (bias, scale, alpha):
            inputs.append(
                mybir.ImmediateValue(dtype=mybir.dt.float32, value=float(arg))
            )
        return sc.add_instruction(
            mybir.InstActivation(
                name=sc.bass.get_next_instruction_name(),
                func=func,
                ins=inputs,
                outs=[sc.lower_ap(out)],
            )
        )

    recip_fn = mybir.ActivationFunctionType.Reciprocal

    with ExitStack() as ctx:
        tc = ctx.enter_context(tile.TileContext(nc))
        consts = ctx.enter_context(tc.tile_pool(name="consts", bufs=1))
        # Two 4-bank PSUM pair-tiles: ACT drains one pair (2 x 1024-col
        # recips) while the PE fills the other -- always opposite PSUM
        # halves (same-half concurrency stretches MMs ~2x, measured v2).
        psA = ctx.enter_context(tc.tile_pool(name="psA", bufs=1, space="PSUM"))
        psB = ctx.enter_context(tc.tile_pool(name="psB", bufs=1, space="PSUM"))
        wresp = ctx.enter_context(tc.tile_pool(name="wresp", bufs=3))
        wot = ctx.enter_context(tc.tile_pool(name="wot", bufs=2))

        xsT = consts.tile([128, R], bf16)
        ysT = consts.tile([128, N], bf16)
        # Aug operand copies at partitions 0:4, 32:36, 64:68, 96:100 --
        # one per PE row-group, so the four 512-col aug streams of a
        # pair run concurrently (one PSUM bank per row-group).
        aug = consts.tile([100, R + N], bf16)
        hci = consts.tile([128, RCHUNKS], f32)
        cjb = consts.tile([128, N], bf16, name="cjb") if apply_gate else None
        scratch = consts.tile([128, 1], f32)

        def load_cjb(q):
            qs = slice(q * PAIRW, (q + 1) * PAIRW)
            src = hcj_d[0:1, qs]
            src_b = bass.AP(
                tensor=src.tensor,
                offset=src.offset,
                ap=[[0, 128], src.ap[-1]],
            )
            nc.sync.dma_start(out=cjb[:, qs], in_=src_b)

        # Input DMA priority. The 16 SDMA engines round-robin between
        # the two HWDGE rings at packet granularity, so putting heavy
        # transfers on both rings halves the rate of the critical one.
        # Therefore: ALL heavy inputs go on the sync ring, serialized in
        # the order the pair loop consumes them (each ring drains FIFO);
        # the scalar ring carries only the tiny tensors and the ACT
        # table preload. ysT in 1024-col chunks (2KB descriptor lines;
        # DMA cost is ~per-descriptor, 1KB lines run at half rate) so
        # pair 0's mains start after ~384KB, not 1.2MB.
        # Consumption-ordered: warmup needs only xsT[:, 0:128] (row
        # chunk 0's lhsT); pair 0 consumes ys[0:2048] in 512-col steps;
        # xsT's remaining rows aren't touched until pair 1 (~+2.8us);
        # cjb0 not until pair 0's first gate (~+2.5us); the q=1 column
        # half has ~11us of slack.
        nc.sync.dma_start(out=xsT[:, 0:128], in_=xsT_d[:, 0:128])
        for q in range(4):
            qs = slice(q * 512, (q + 1) * 512)
            nc.sync.dma_start(out=ysT[:, qs], in_=ysT_d[:, qs])
        nc.sync.dma_start(out=xsT[:, 128:512], in_=xsT_d[:, 128:512])
        if apply_gate:
            load_cjb(0)
        nc.sync.dma_start(out=ysT[:, 2048:3072], in_=ysT_d[:, 2048:3072])
        nc.sync.dma_start(out=ysT[:, 3072:4096], in_=ysT_d[:, 3072:4096])
        if apply_gate:
            load_cjb(1)
        nc.scalar.dma_start(out=hci, in_=hci_d[:, :])
        # ACT spline-table preload: dummy 1-element Reciprocal on hci
        # (lands first on the scalar ring). Walrus hoists the ~2.7us
        # ACT_TABLE_LOAD+drain in front of it, inside the input-DMA
        # window; the load DMA runs concurrently with the ring's
        # DIRECT2D descriptor-gen.
        raw_activation(scratch, hci[:, 0:1], recip_fn)
        for g in range(4):
            nc.scalar.dma_start(out=aug[32 * g : 32 * g + 4, :], in_=aug_d[:, :])

        # PE warmup off xsT into a dead PSUM slot: bridges the gap from
        # xsT arrival to ys[0:512] arrival so the PE has no activity
        # gap (a gap drops the HAM clock a notch -- measured v2).
        wp = psA.tile([128, PAIRW], f32, tag="d")
        for _ in range(10):
            nc.tensor.matmul(
                wp[:, 0:128],
                lhsT=xsT[:, 0:128],
                rhs=xsT[:, 0:128],
                start=True,
                stop=True,
            )

        # Pair order: q-outer snake -- all four row chunks consume
        # column half q=0 first, so the first ~11us of compute needs
        # only ysT[:, 0:2048] + cjb[:, 0:2048]; the second column half
        # has 11us of slack to load. Within a pair the 4 mains share
        # one LDWEIGHTS and the aug pass is one 2-way concurrent
        # 2048-col sweep.
        order = [(r, 0) for r in range(RCHUNKS)]
        order += [(r, 1) for r in reversed(range(RCHUNKS))]

        for p, (r, q) in enumerate(order):
            rs = slice(r * 128, (r + 1) * 128)
            base = q * PAIRW
            pool = psA if p % 2 == 0 else psB
            pd = pool.tile([128, PAIRW], f32, tag="d")
            # Per-pair output staging: the pair's 2 gates write one
            # [128, 2048] SBUF tile, flushed by ONE DMA with 4KB
            # contiguous DRAM lines. Per-tile [128, 1024] stores have
            # 2KB lines and are descriptor-bound (~245GB/s); 4MB of
            # output at that rate trails the compute by >10us.
            ot_full = wot.tile([128, PAIRW], bf16, tag="ot")
            # 4 mains, one shared lhsT (single LDWEIGHTS per pair).
            for h in range(4):
                cs = slice(base + h * 512, base + (h + 1) * 512)
                ps = slice(h * 512, (h + 1) * 512)
                nc.tensor.matmul(
                    pd[:, ps],
                    lhsT=xsT[:, rs],
                    rhs=ysT[:, cs],
                    start=True,
                    stop=False,
                )
            # Aug pass: four 512-col streams at PE row-groups 0/32/64/96,
            # one per PSUM bank of the pair, all concurrent -- the pair's
            # whole 2048-col aug sweep walls ~512 cycles + one LDWEIGHTS
            # (the later row-groups' LDW+stream hide inside the first
            # stream, as measured for the 2-way variant).
            for b in range(4):
                rg = 32 * b
                kw = {} if rg == 0 else {"tile_position": (rg, 0)}
                nc.tensor.matmul(
                    pd[:, b * 512 : (b + 1) * 512],
                    lhsT=aug[rg : rg + 4, rs],
                    rhs=aug[
                        rg : rg + 4,
                        R + base + b * 512 : R + base + (b + 1) * 512,
                    ],
                    start=False,
                    stop=True,
                    **kw,
                )
            # Drain in 1024-col halves: ACT recip -> DVE gate into the
            # row-chunk staging tile.
            for h in (0, 1):
                hs = slice(h * W, (h + 1) * W)
                cs = slice(base + h * W, base + (h + 1) * W)
                resp = wresp.tile([128, W], bf16, tag="resp")
                raw_activation(resp, pd[:, hs], recip_fn)
                if apply_gate:
                    nc.vector._custom_dve(
                        gate_op,
                        out=ot_full[:, hs],
                        in0=resp,
                        in1=cjb[:, cs],
                        s0=hci[:, r : r + 1],
                        s1=float(16.0 * A3),
                        imm2=float(4.0 * A1),
                    )
                else:
                    # no-gate: res = 2*res' via a second ACT pass on PSUM
                    raw_activation(ot_full[:, hs], pd[:, hs], recip_fn, scale=0.5)
            nc.sync.dma_start(
                out=out_d[rs, base : base + PAIRW], in_=ot_full
            )

    nc.finalize()
    return nc


def kernel(x, y, sample_x, sample_y, scale, cutoff):
    import ml_dtypes

    from concourse.bass_utils import run_bass_kernel_spmd

    f32 = np.float32
    bf16 = ml_dtypes.bfloat16

    # Host prep in float64 for accuracy, cast down for the device.
    x64 = np.asarray(x, np.float64)
    y64 = np.asarray(y, np.float64)
    s64 = np.clip(np.asarray(scale, np.float64), 1e-6, 1e6)
    scale_x = np.clip(np.asarray(sample_x, np.float64) @ s64, 1e-6, None)
    scale_y = np.clip(np.asarray(sample_y, np.float64) @ s64, 1e-6, None)
    x_s = (x64 / np.sqrt(scale_x)).astype(f32)          # [N, D]
    y_s = (y64 / np.sqrt(scale_y)).astype(f32)          # [N, D]
    # Norms from the bf16-rounded operands the PE will actually multiply,
    # so the x2/y2 terms match the -4xy term's operand rounding.
    x_sb = x_s.astype(bf16)
    y_sb = y_s.astype(bf16)
    x2 = np.sum(x_sb.astype(np.float64) ** 2, axis=1)   # [N]
    y2 = np.sum(y_sb.astype(np.float64) ** 2, axis=1)   # [N]

    # PSUM carries 2(1+d): -4xy via ysT scale, doubled norms via aug rows.
    ysT = np.ascontiguousarray((-4.0 * y_sb.astype(np.float64)).T).astype(bf16)
    xsT_full = np.ascontiguousarray(x_sb.T)                      # [128, N] bf16
    y2p2 = 2.0 * y2 + 2.0
    yh = y2p2.astype(bf16)
    yl = (y2p2 - yh.astype(np.float64)).astype(bf16)
    ones_n = np.ones(N, np.float64)
    augy = np.stack(
        [ones_n, ones_n, yh.astype(np.float64), yl.astype(np.float64)]
    )                                                            # [4, N]
    x2_2 = 2.0 * x2
    x2h = x2_2.astype(bf16)
    x2l = (x2_2 - x2h.astype(np.float64)).astype(bf16)
    # gate op takes c/4: t' = res' - (c_j/4 + c_i/4) = (res - cm)/2
    c_q = 0.25 * np.clip(np.asarray(cutoff, np.float64), 1e-4, 0.9999)
    hcj = np.ascontiguousarray(c_q.reshape(1, N)).astype(bf16)     # [1, N]

    apply_gate = bool(np.mean(np.asarray(cutoff, np.float64)) > 0.0)

    key = apply_gate
    if key not in _PROGRAM_CACHE:
        _PROGRAM_CACHE[key] = _build_program(apply_gate)
    nc = _PROGRAM_CACHE[key]

    in_maps = []
    for i in range(NCORES):
        rows = slice(i * R, (i + 1) * R)
        ones_r = np.ones(R, np.float64)
        augx = np.stack(
            [x2h.astype(np.float64)[rows], x2l.astype(np.float64)[rows],
             ones_r, ones_r]
        )                                                        # [4, R]
        aug = np.ascontiguousarray(
            np.concatenate([augx, augy], axis=1)
        ).astype(bf16)                                           # [4, R+N]
        hci = np.ascontiguousarray(
            c_q[rows, 0].reshape(RCHUNKS, 128).T, dtype=f32
        )                                                        # [128, RCHUNKS]
        in_maps.append(
            {
                "xsT": np.ascontiguousarray(xsT_full[:, rows]),
                "ysT": ysT,
                "aug": aug,
                "hci": hci,
                "hcj": hcj,
            }
        )

    out = run_bass_kernel_spmd(nc, in_maps, list(range(NCORES)))
    full = np.concatenate(
        [np.asarray(out.results[i]["out"]) for i in range(NCORES)], axis=0
    )
    return np.ascontiguousarray(full.astype(f32))


# revision 16
# speedup vs baseline: 1.0629x; 1.0629x over previous
"""Trainium2 Bass kernel for BatchedCauchyKernel_CONCERT_flex (v3).

Full-input contract: kernel(**inputs) takes the complete (unsharded)
numpy arrays, shards x/sample_x/cutoff rows across 8 NeuronCores
(data-parallel over the N axis of the output), replicates y/sample_y/
scale, and gathers the per-core [512, 4096] tiles into the full
[4096, 4096] output.

Math (reference):
    s        = clip(scale, 1e-6, 1e6)
    scale_x  = clip(sample_x @ s, 1e-6)        x_s = x / sqrt(scale_x)
    scale_y  = clip(sample_y @ s, 1e-6)        y_s = y / sqrt(scale_y)
    d        = clip(|x_s_i|^2 + |y_s_j|^2 - 2 x_s_i . y_s_j, 1e-6)
    res      = 1 / (1 + d)
    c        = clip(cutoff, 1e-4, 0.9999)
    cm_ij    = (c_i + c_j) / 2
    out      = res * sigmoid(clip(res - cm, -1, 1))     (iff mean(cutoff) > 0)

Device formulation (per core) -- v2's math, v3's schedule:
    PSUM = 2 + 2|x_i|^2 + 2|y_j|^2 - 4 x.y = 2(1+d)  via accumulating
    matmuls (bf16 mains K=128 vs -4*ysT; split-bf16 aug K=4 for the
    doubled norm terms; two aug streams run concurrently at PE
    row-groups 0 and 32 via tile_position).
    ACT:  res' = Reciprocal(PSUM) -> bf16          [res' = res/2]
    DVE:  fused CAUCHY_GATE_ANT op (8 ALU nodes, 1 elem/cycle):
          t'  = Src0 - (Src1 + C0)                 [= (res - (c_j+c_i)/2)/2]
          out = Src0 * (One + t'*(C2 + C1*t'^2))   [cubic sigmoid fit]

v3 schedule changes (driven by the v2 trace; all windows in exec-time):
    - v2 spent ~25us of serialized DIRECT2D descriptor-gen on the Sync
      sequencer (41 dma_starts x ~605ns) -- input loading could not even
      begin until ~2.3us into the window and the PE sat idle for the
      first 6us. v3 cuts input dma_starts 25 -> 7 (2048-col ysT/cjb
      chunks = 4KB descriptor lines; aug tensors packed into one dram
      param loaded twice) and splits them across BOTH HWDGE rings
      (nc.scalar + nc.sync), with the 16 output DMAs alone on Sync.
    - PSUM is organized as two 4-bank pair-tiles [128, 2048] (pools
      psA/psB) instead of four 2-bank tiles: per pair, the 4 mains
      share one xsT LDWEIGHTS and the two K=4 aug streams cover 2048
      cols in one 2-way concurrent pass (was: LDW + aug pass per 1024
      cols). Steady-state PE cost drops ~1507 -> ~1340ns per 1024 cols
      at the DMA-throttled 1.2 GHz PE clock. ACT reads one pair while
      the PE fills the other (opposite PSUM halves, as in v2).
    - Epilogue still drains in 1024-col halves (ACT recip -> DVE gate
      -> out DMA) so ACT/DVE stay under the PE pace and the last-tile
      drain stays short.
    - Warmup is 16 x 128-col MMs off xsT (lands first) bridging the gap
      until ysT chunk 0 arrives -- keeps the PE continuously busy from
      ~2us into the window (v2: first MM at ~6us).
    - ACT spline-table preload via a dummy 1-element Reciprocal on xsT
      right after the ysT chunk-0 descriptor is issued.
    - Pool buffers cut 18 -> 9 (wresp/wot 6 -> 3) to shrink the
      end-of-program teardown (drain + barriers + dma_reset/sem_clear
      scale with allocated sems; v2 spent ~9us there).
The row scaling / row norms (O(N*D), 0.025% of the FLOPs) are host prep.
"""

from __future__ import annotations

import numpy as np

N = 4096
D = 128
S = 16
NCORES = 8
R = N // NCORES          # 512 rows of x per core
RCHUNKS = R // 128       # 4 row chunks of 128 (PSUM partition dim)
W = 1024                 # epilogue tile width (2 PSUM banks)
PAIRW = 2048             # PSUM pair-tile width (4 banks)

# sigmoid(t) ~= 0.5 + A1*t + A3*t^3, minimax on t in [-1, 0.05] (err 1.2e-4)
A1 = 0.24939704
A3 = -0.01842716

_PROGRAM_CACHE = {}
_GATE_OP = []


def _register_gate_op():
    """Register the fused gate op in concourse.dve_ops' tables (the
    documented extension point is appending to OPS; the per-NEFF DVE
    table is generated from OPS by name). Idempotent."""
    import concourse.dve_ops as dops
    from concourse.dve_spec import Spec, Src0, Src1, C0, C1, C2, One
    from concourse.dve_uop import DveOpSpec

    if _GATE_OP:
        return _GATE_OP[0]
    name = "CAUCHY_GATE_ANT"
    for op in dops.OPS:
        if op.name == name:
            _GATE_OP.append(op)
            return op

    s = Src1 + C0
    t = Src0 - s
    w = One + t * (C2 + C1 * (t * t))
    body = w * Src0

    def ref(in0, in1, c0, c1, c2):
        i0 = in0.astype(np.float32)
        tt = i0 - (in1.astype(np.float32) + c0)
        return (i0 * (1.0 + tt * (c2 + c1 * (tt * tt)))).astype(np.float32)

    spec = Spec(body=body, reference=ref)
    row = dops._CUSTOM_DVE_ROW_BASE + len(dops.OPS)
    assert row < 0x20
    tmp = DveOpSpec(
        name=name, opcode=row, uops=dops.lower(spec, ver="v3"), rd1_en=True
    )
    op = dops.DveOp(name, spec, subdim=False, uops_sha={"v3": tmp.sha("v3")})
    dops.OPS.append(op)
    dops._SUB_OPCODE_FOR_NAME[name] = row
    dops.CUSTOM_DVE_SPECS[name] = spec
    _GATE_OP.append(op)
    return op


def _build_program(apply_gate: bool):
    from contextlib import ExitStack

    import concourse.bass as bass
    import concourse.tile as tile
    from concourse import bacc, mybir

    f32 = mybir.dt.float32
    bf16 = mybir.dt.bfloat16
    gate_op = _register_gate_op()

    nc = bacc.Bacc()

    xsT_d = nc.declare_dram_parameter("xsT", [128, R], bf16, isOutput=False)
    ysT_d = nc.declare_dram_parameter("ysT", [128, N], bf16, isOutput=False)
    # aug rows k over cols [x-part(0:R) | y-part(R:R+N)]:
    #   k=0: [x2h | 1],  k=1: [x2l | 1],  k=2: [1 | yh],  k=3: [1 | yl]
    # so sum_k augx[k,i]*augy[k,j] = 2*x2_i + 2*y2_j + 2.
    aug_d = nc.declare_dram_parameter("aug", [4, R + N], bf16, isOutput=False)
    hci_d = nc.declare_dram_parameter("hci", [128, RCHUNKS], f32, isOutput=False)
    hcj_d = nc.declare_dram_parameter("hcj", [1, N], bf16, isOutput=False)
    out_d = nc.declare_dram_parameter("out", [R, N], bf16, isOutput=True)

    def raw_activation(out, in_, func, bias=0.0, scale=1.0, alpha=0.0):
        sc = nc.scalar
        inputs = [sc.lower_ap(in_)]
        for arg in (bias, scale, alpha):
            inputs.append(
                mybir.ImmediateValue(dtype=mybir.dt.float32, value=float(arg))
            )
        return sc.add_instruction(
            mybir.InstActivation(
                name=sc.bass.get_next_instruction_name(),
                func=func,
                ins=inputs,
                outs=[sc.lower_ap(out)],
            )
        )

    recip_fn = mybir.ActivationFunctionType.Reciprocal

    with ExitStack() as ctx:
        tc = ctx.enter_context(tile.TileContext(nc))
        consts = ctx.enter_context(tc.tile_pool(name="consts", bufs=1))
        # Two 4-bank PSUM pair-tiles: ACT drains one pair (2 x 1024-col
        # recips) while the PE fills the other -- always opposite PSUM
        # halves (same-half concurrency stretches MMs ~2x, measured v2).
        psA = ctx.enter_context(tc.tile_pool(name="psA", bufs=1, space="PSUM"))
        psB = ctx.enter_context(tc.tile_pool(name="psB", bufs=1, space="PSUM"))
        wresp = ctx.enter_context(tc.tile_pool(name="wresp", bufs=5))
        wot = ctx.enter_context(tc.tile_pool(name="wot", bufs=3))

        xsT = consts.tile([128, R], bf16)
        ysT = consts.tile([128, N], bf16)
        # Aug operand copies at partitions 0:4, 32:36, 64:68, 96:100 --
        # one per PE row-group, so the four 512-col aug streams of a
        # pair run concurrently (one PSUM bank per row-group).
        aug = consts.tile([100, R + N], bf16)
        hci = consts.tile([128, RCHUNKS], f32)
        cjb = consts.tile([128, N], bf16, name="cjb") if apply_gate else None
        scratch = consts.tile([128, 1], f32)

        def load_cjb(q):
            qs = slice(q * PAIRW, (q + 1) * PAIRW)
            src = hcj_d[0:1, qs]
            src_b = bass.AP(
                tensor=src.tensor,
                offset=src.offset,
                ap=[[0, 128], src.ap[-1]],
            )
            nc.sync.dma_start(out=cjb[:, qs], in_=src_b)

        # Input DMA priority. The 16 SDMA engines round-robin between
        # the two HWDGE rings at packet granularity, so putting heavy
        # transfers on both rings halves the rate of the critical one.
        # Therefore: ALL heavy inputs go on the sync ring, serialized in
        # the order the pair loop consumes them (each ring drains FIFO);
        # the scalar ring carries only the tiny tensors and the ACT
        # table preload. ysT in 1024-col chunks (2KB descriptor lines;
        # DMA cost is ~per-descriptor, 1KB lines run at half rate) so
        # pair 0's mains start after ~384KB, not 1.2MB.
        # Consumption-ordered: warmup needs only xsT[:, 0:128] (row
        # chunk 0's lhsT); pair 0 consumes ys[0:2048] in 512-col steps;
        # xsT's remaining rows aren't touched until pair 1 (~+2.8us);
        # cjb0 not until pair 0's first gate (~+2.5us); the q=1 column
        # half has ~11us of slack.
        nc.sync.dma_start(out=xsT[:, 0:128], in_=xsT_d[:, 0:128])
        for q in range(4):
            qs = slice(q * 512, (q + 1) * 512)
            nc.sync.dma_start(out=ysT[:, qs], in_=ysT_d[:, qs])
        nc.sync.dma_start(out=xsT[:, 128:512], in_=xsT_d[:, 128:512])
        if apply_gate:
            load_cjb(0)
        nc.sync.dma_start(out=ysT[:, 2048:3072], in_=ysT_d[:, 2048:3072])
        nc.sync.dma_start(out=ysT[:, 3072:4096], in_=ysT_d[:, 3072:4096])
        if apply_gate:
            load_cjb(1)
        # Scalar ring: aug copies FIRST (needed ~1.8us after mains
        # start; and the dummy recip below BLOCKS this ring until its
        # input lands + the table loads), then hci, then the preload.
        for g in range(4):
            nc.scalar.dma_start(out=aug[32 * g : 32 * g + 4, :], in_=aug_d[:, :])
        nc.scalar.dma_start(out=hci, in_=hci_d[:, :])
        # ACT spline-table preload: dummy 1-element Reciprocal on hci.
        # Walrus hoists the ~2.7us ACT_TABLE_LOAD+drain in front of it,
        # inside the input-DMA window; the load DMA runs concurrently
        # with the ring's DIRECT2D descriptor-gen.
        raw_activation(scratch, hci[:, 0:1], recip_fn)

        # PE warmup off xsT into a dead PSUM slot: bridges the gap from
        # xsT arrival to ys[0:512] arrival so the PE has no activity
        # gap (a gap drops the HAM clock a notch -- measured v2).
        wp = psA.tile([128, PAIRW], f32, tag="d")
        for _ in range(10):
            nc.tensor.matmul(
                wp[:, 0:128],
                lhsT=xsT[:, 0:128],
                rhs=xsT[:, 0:128],
                start=True,
                stop=True,
            )

        # Pair order: q-outer snake -- all four row chunks consume
        # column half q=0 first, so the first ~11us of compute needs
        # only ysT[:, 0:2048] + cjb[:, 0:2048]; the second column half
        # has 11us of slack to load. Within a pair the 4 mains share
        # one LDWEIGHTS and the aug pass is one 2-way concurrent
        # 2048-col sweep.
        order = [(r, 0) for r in range(RCHUNKS)]
        order += [(r, 1) for r in reversed(range(RCHUNKS))]

        for p, (r, q) in enumerate(order):
            rs = slice(r * 128, (r + 1) * 128)
            base = q * PAIRW
            pool = psA if p % 2 == 0 else psB
            pd = pool.tile([128, PAIRW], f32, tag="d")
            # Per-pair output staging: the pair's 2 gates write one
            # [128, 2048] SBUF tile, flushed by ONE DMA with 4KB
            # contiguous DRAM lines. Per-tile [128, 1024] stores have
            # 2KB lines and are descriptor-bound (~245GB/s); 4MB of
            # output at that rate trails the compute by >10us.
            ot_full = wot.tile([128, PAIRW], bf16, tag="ot")
            # 4 mains, one shared lhsT (single LDWEIGHTS per pair).
            for h in range(4):
                cs = slice(base + h * 512, base + (h + 1) * 512)
                ps = slice(h * 512, (h + 1) * 512)
                nc.tensor.matmul(
                    pd[:, ps],
                    lhsT=xsT[:, rs],
                    rhs=ysT[:, cs],
                    start=True,
                    stop=False,
                )
            # Aug pass: four 512-col streams at PE row-groups 0/32/64/96,
            # one per PSUM bank of the pair, all concurrent -- the pair's
            # whole 2048-col aug sweep walls ~512 cycles + one LDWEIGHTS
            # (the later row-groups' LDW+stream hide inside the first
            # stream, as measured for the 2-way variant).
            for b in range(4):
                rg = 32 * b
                kw = {} if rg == 0 else {"tile_position": (rg, 0)}
                nc.tensor.matmul(
                    pd[:, b * 512 : (b + 1) * 512],
                    lhsT=aug[rg : rg + 4, rs],
                    rhs=aug[
                        rg : rg + 4,
                        R + base + b * 512 : R + base + (b + 1) * 512,
                    ],
                    start=False,
                    stop=True,
                    **kw,
                )
            # Drain in 1024-col halves: ACT recip -> DVE gate into the
            # row-chunk staging tile.
            for h in (0, 1):
                hs = slice(h * W, (h + 1) * W)
                cs = slice(base + h * W, base + (h + 1) * W)
                resp = wresp.tile([128, W], bf16, tag="resp")
                raw_activation(resp, pd[:, hs], recip_fn)
                if apply_gate:
                    nc.vector._custom_dve(
                        gate_op,
                        out=ot_full[:, hs],
                        in0=resp,
                        in1=cjb[:, cs],
                        s0=hci[:, r : r + 1],
                        s1=float(16.0 * A3),
                        imm2=float(4.0 * A1),
                    )
                else:
                    # no-gate: res = 2*res' via a second ACT pass on PSUM
                    raw_activation(ot_full[:, hs], pd[:, hs], recip_fn, scale=0.5)
            nc.sync.dma_start(
                out=out_d[rs, base : base + PAIRW], in_=ot_full
            )

    nc.finalize()
    return nc


def kernel(x, y, sample_x, sample_y, scale, cutoff):
    import ml_dtypes

    from concourse.bass_utils import run_bass_kernel_spmd

    f32 = np.float32
    bf16 = ml_dtypes.bfloat16

    # Host prep in float64 for accuracy, cast down for the device.
    x64 = np.asarray(x, np.float64)
    y64 = np.asarray(y, np.float64)
    s64 = np.clip(np.asarray(scale, np.float64), 1e-6, 1e6)
    scale_x = np.clip(np.asarray(sample_x, np.float64) @ s64, 1e-6, None)
    scale_y = np.clip(np.asarray(sample_y, np.float64) @ s64, 1e-6, None)
    x_s = (x64 / np.sqrt(scale_x)).astype(f32)          # [N, D]
    y_s = (y64 / np.sqrt(scale_y)).astype(f32)          # [N, D]
    # Norms from the bf16-rounded operands the PE will actually multiply,
    # so the x2/y2 terms match the -4xy term's operand rounding.
    x_sb = x_s.astype(bf16)
    y_sb = y_s.astype(bf16)
    x2 = np.sum(x_sb.astype(np.float64) ** 2, axis=1)   # [N]
    y2 = np.sum(y_sb.astype(np.float64) ** 2, axis=1)   # [N]

    # PSUM carries 2(1+d): -4xy via ysT scale, doubled norms via aug rows.
    ysT = np.ascontiguousarray((-4.0 * y_sb.astype(np.float64)).T).astype(bf16)
    xsT_full = np.ascontiguousarray(x_sb.T)                      # [128, N] bf16
    y2p2 = 2.0 * y2 + 2.0
    yh = y2p2.astype(bf16)
    yl = (y2p2 - yh.astype(np.float64)).astype(bf16)
    ones_n = np.ones(N, np.float64)
    augy = np.stack(
        [ones_n, ones_n, yh.astype(np.float64), yl.astype(np.float64)]
    )                                                            # [4, N]
    x2_2 = 2.0 * x2
    x2h = x2_2.astype(bf16)
    x2l = (x2_2 - x2h.astype(np.float64)).astype(bf16)
    # gate op takes c/4: t' = res' - (c_j/4 + c_i/4) = (res - cm)/2
    c_q = 0.25 * np.clip(np.asarray(cutoff, np.float64), 1e-4, 0.9999)
    hcj = np.ascontiguousarray(c_q.reshape(1, N)).astype(bf16)     # [1, N]

    apply_gate = bool(np.mean(np.asarray(cutoff, np.float64)) > 0.0)

    key = apply_gate
    if key not in _PROGRAM_CACHE:
        _PROGRAM_CACHE[key] = _build_program(apply_gate)
    nc = _PROGRAM_CACHE[key]

    in_maps = []
    for i in range(NCORES):
        rows = slice(i * R, (i + 1) * R)
        ones_r = np.ones(R, np.float64)
        augx = np.stack(
            [x2h.astype(np.float64)[rows], x2l.astype(np.float64)[rows],
             ones_r, ones_r]
        )                                                        # [4, R]
        aug = np.ascontiguousarray(
            np.concatenate([augx, augy], axis=1)
        ).astype(bf16)                                           # [4, R+N]
        hci = np.ascontiguousarray(
            c_q[rows, 0].reshape(RCHUNKS, 128).T, dtype=f32
        )                                                        # [128, RCHUNKS]
        in_maps.append(
            {
                "xsT": np.ascontiguousarray(xsT_full[:, rows]),
                "ysT": ysT,
                "aug": aug,
                "hci": hci,
                "hcj": hcj,
            }
        )

    out = run_bass_kernel_spmd(nc, in_maps, list(range(NCORES)))
    full = np.concatenate(
        [np.asarray(out.results[i]["out"]) for i in range(NCORES)], axis=0
    )
    return np.ascontiguousarray(full.astype(f32))
